# revision 1
# baseline (speedup 1.0000x reference)
"""GCN (3-layer) + graph pooling kernel for 8 Trainium2 NeuronCores.

Sharding: each core owns 64 of the 512 graphs (contiguous node range, since
`batch` is sorted). Every graph gets a fixed SLOT=240-row slot so per-core
layouts are uniform (SPMD). Edges are assigned to the core owning their dst;
gathers read a replicated (AllGather'd) feature table with int16 indices
(4 src buckets of 2 cores each keep indices < 32768); scatter-adds are split
into rounds with unique dst per instruction (the DMA RMW races on duplicate
indices within one instruction).
"""
import numpy as np

import concourse.bacc as bacc
import concourse.bass as bass
import concourse.mybir as mybir
import concourse.tile as tile
from concourse.bass_utils import run_bass_kernel_spmd

LAST_EXEC_NS = None
LAST_RESULT = None

F = 64          # padded feature width (layer1's 32 is zero-padded to 64)
NC = 8          # cores
NEG_BIG = -1.0e30


def _pack_idx(flat: np.ndarray) -> np.ndarray:
    """token i -> [i % 16, i // 16], replicated down to 128 partitions."""
    t = len(flat)
    a = np.ascontiguousarray(flat.astype(np.int16).reshape(t // 16, 16).T)
    return np.tile(a, (8, 1))


def _build_nc(R, ntiles, segments, GPC, SLOT, TOT):
    """segments: list of (bucket, n_tokens) in stream order; n_tokens % 128 == 0."""
    import os
    PHASES = int(os.environ.get("GCN_PHASES", "3"))  # 1=dense only, 2=+edges, 3=full
    AGG_ROWS = R + 128
    NGRP = ntiles // 15          # groups of 15 tiles == 8 graph slots
    nc = bacc.Bacc("TRN2", target_bir_lowering=False, debug=False, num_devices=NC)
    dt = mybir.dt
    AF = mybir.ActivationFunctionType

    # ---- external inputs ----
    t_xT = nc.dram_tensor("xT", [4, R], dt.float32, kind="ExternalInput")
    t_gidx = nc.dram_tensor("gidx", [128, TOT // 16], dt.int16, kind="ExternalInput")
    t_sidx = nc.dram_tensor("sidx", [128, TOT // 16], dt.int16, kind="ExternalInput")
    t_dinv = nc.dram_tensor("dinv", [128, ntiles], dt.float32, kind="ExternalInput")
    t_mask = nc.dram_tensor("mask", [128, ntiles], dt.float32, kind="ExternalInput")
    t_negm = nc.dram_tensor("negm", [128, ntiles], dt.float32, kind="ExternalInput")
    t_sel = nc.dram_tensor("sel", [128, 15, 8], dt.float32, kind="ExternalInput")
    t_cntinv = nc.dram_tensor("cntinv", [128, GPC], dt.float32, kind="ExternalInput")
    t_w1 = nc.dram_tensor("w1", [4, F], dt.float32, kind="ExternalInput")
    t_w2 = nc.dram_tensor("w2", [F, F], dt.float32, kind="ExternalInput")
    t_w3 = nc.dram_tensor("w3", [F, F], dt.float32, kind="ExternalInput")
    t_wc = nc.dram_tensor("wc", [128, 2], dt.float32, kind="ExternalInput")
    t_b1 = nc.dram_tensor("b1r", [128, F], dt.float32, kind="ExternalInput")
    t_b2 = nc.dram_tensor("b2r", [128, F], dt.float32, kind="ExternalInput")
    t_b3 = nc.dram_tensor("b3r", [128, F], dt.float32, kind="ExternalInput")
    t_bc = nc.dram_tensor("bcr", [GPC, 2], dt.float32, kind="ExternalInput")
    t_id = nc.dram_tensor("ident", [128, 128], dt.float32, kind="ExternalInput")
    t_out = nc.dram_tensor("out", [GPC, 2], dt.float32, kind="ExternalOutput")

    # ---- internal DRAM ----
    t_bounce = [nc.dram_tensor(f"bounce{l}", [R, F], dt.float32) for l in range(3)]
    t_pfull = [nc.dram_tensor(f"pfull{l}", [NC * R, F], dt.float32,
                              addr_space="Shared") for l in range(3)]
    t_agg = [nc.dram_tensor(f"agg{l}", [AGG_ROWS, F], dt.float32) for l in range(3)]

    with tile.TileContext(nc) as tc:
        with (
            tc.tile_pool(name="persist", bufs=1) as pc,
            tc.tile_pool(name="dense", bufs=3) as dp,
            tc.tile_pool(name="edge", bufs=2) as ep,
            tc.tile_pool(name="psum", bufs=2, space="PSUM") as pp,
            tc.tile_pool(name="psacc", bufs=1, space="PSUM") as pa,
        ):
            # ---- persistent small tiles ----
            xt_sb = pc.tile([4, R], dt.float32, tag="xt")
            dinv_sb = pc.tile([128, ntiles], dt.float32, tag="dinv")
            mask_sb = pc.tile([128, ntiles], dt.float32, tag="mask")
            negm_sb = pc.tile([128, ntiles], dt.float32, tag="negm")
            sel_sb = pc.tile([128, 15, 8], dt.float32, tag="sel")
            cntinv_sb = pc.tile([128, GPC], dt.float32, tag="cntinv")
            w1_sb = pc.tile([4, F], dt.float32, tag="w1")
            w2_sb = pc.tile([F, F], dt.float32, tag="w2")
            w3_sb = pc.tile([F, F], dt.float32, tag="w3")
            wc_sb = pc.tile([128, 2], dt.float32, tag="wc")
            b1_sb = pc.tile([128, F], dt.float32, tag="b1")
            b2_sb = pc.tile([128, F], dt.float32, tag="b2")
            b3_sb = pc.tile([128, F], dt.float32, tag="b3")
            bc_sb = pc.tile([GPC, 2], dt.float32, tag="bc")
            id_sb = pc.tile([128, 128], dt.float32, tag="ident")
            h3maxT = pc.tile([F, R], dt.float32, tag="h3maxT")
            combT = pc.tile([128, GPC], dt.float32, tag="combT")

            nc.sync.dma_start(xt_sb[:], t_xT.ap())
            nc.sync.dma_start(dinv_sb[:], t_dinv.ap())
            nc.sync.dma_start(mask_sb[:], t_mask.ap())
            nc.sync.dma_start(negm_sb[:], t_negm.ap())
            nc.sync.dma_start(sel_sb[:], t_sel.ap())
            nc.sync.dma_start(cntinv_sb[:], t_cntinv.ap())
            nc.sync.dma_start(w1_sb[:], t_w1.ap())
            nc.sync.dma_start(w2_sb[:], t_w2.ap())
            nc.sync.dma_start(w3_sb[:], t_w3.ap())
            nc.sync.dma_start(wc_sb[:], t_wc.ap())
            nc.sync.dma_start(b1_sb[:], t_b1.ap())
            nc.sync.dma_start(b2_sb[:], t_b2.ap())
            nc.sync.dma_start(b3_sb[:], t_b3.ap())
            nc.sync.dma_start(bc_sb[:], t_bc.ap())
            nc.sync.dma_start(id_sb[:], t_id.ap())

            w_for = {1: w2_sb, 2: w3_sb}
            b_for = {1: b1_sb, 2: b2_sb}

            # ---------------- dense phase of layer l (produce P_l, init agg_l)
            def dense_layer(l):
                for m in range(ntiles):
                    ts = slice(m * 128, (m + 1) * 128)
                    dcol = dinv_sb[:, m:m + 1]
                    if l == 0:
                        mm = pp.tile([128, F], dt.float32, tag="mm")
                        nc.tensor.matmul(mm[:], xt_sb[0:3, ts], w1_sb[0:3, :],
                                         start=True, stop=True)
                    else:
                        at = dp.tile([128, F], dt.float32, tag="at")
                        nc.sync.dma_start(at[:], t_agg[l - 1].ap()[ts])
                        z = dp.tile([128, F], dt.float32, tag="z")
                        nc.vector.tensor_scalar_mul(z[:], at[:], dcol)
                        nc.vector.tensor_add(z[:], z[:], b_for[l][:])
                        zr = dp.tile([128, F], dt.float32, tag="zr")
                        nc.scalar.activation(zr[:], z[:], AF.Relu)
                        ztp = pp.tile([F, 128], dt.float32, tag="ztp")
                        nc.tensor.transpose(ztp[:], zr[:], id_sb[:])
                        zts = dp.tile([F, 128], dt.float32, tag="zts")
                        nc.scalar.copy(zts[:], ztp[:])
                        mm = pp.tile([128, F], dt.float32, tag="mm")
                        nc.tensor.matmul(mm[:], zts[:], w_for[l][:],
                                         start=True, stop=True)
                    ps = dp.tile([128, F], dt.float32, tag="ps")
                    nc.scalar.activation(ps[:], mm[:], AF.Copy, scale=dcol)
                    nc.sync.dma_start(t_bounce[l].ap()[ts], ps[:])
                    nc.sync.dma_start(t_agg[l].ap()[ts], ps[:])
                nc.gpsimd.collective_compute(
                    "AllGather", mybir.AluOpType.bypass,
                    replica_groups=[list(range(NC))],
                    ins=[t_bounce[l].ap().opt()],
                    outs=[t_pfull[l].ap().opt()],
                )

            # ---------------- edge phase of layer l (gather + scatter rounds)
            def edge_layer(l):
                off = 0
                for (bkt, n) in segments:
                    o16 = off // 16
                    gi = ep.tile([128, n // 16], dt.int16, tag="gi")
                    si = ep.tile([128, n // 16], dt.int16, tag="si")
                    nc.sync.dma_start(gi[:], t_gidx.ap()[:, o16:o16 + n // 16])
                    nc.sync.dma_start(si[:], t_sidx.ap()[:, o16:o16 + n // 16])
                    gt = ep.tile([128, n // 128, F], dt.float32, tag="gt")
                    src = t_pfull[l].ap()[bkt * 2 * R:(bkt + 1) * 2 * R]
                    nc.gpsimd.dma_gather(gt[:], src, gi[:], n, n, F)
                    nc.gpsimd.dma_scatter_add(t_agg[l].ap(), gt[:], si[:], n, n, F)
                    off += n

            for l in range(3):
                dense_layer(l)
                if PHASES >= 2:
                    edge_layer(l)

            # ---------------- pooling + classifier ----------------
            do_pool = PHASES >= 3
            gmean = pa.tile([F, GPC], dt.float32, tag="gmean")
            pool_tiles = range(ntiles) if do_pool else range(2)
            for m in pool_tiles:
                ts = slice(m * 128, (m + 1) * 128)
                at = dp.tile([128, F], dt.float32, tag="at")
                nc.sync.dma_start(at[:], t_agg[2].ap()[ts])
                h = dp.tile([128, F], dt.float32, tag="z")
                nc.vector.tensor_scalar_mul(h[:], at[:], dinv_sb[:, m:m + 1])
                nc.vector.tensor_add(h[:], h[:], b3_sb[:])
                # sum/mean pool via matmul with slot-selection matrix
                hs = dp.tile([128, F], dt.float32, tag="hs")
                nc.vector.tensor_scalar_mul(hs[:], h[:], mask_sb[:, m:m + 1])
                g = m // 15
                nc.tensor.matmul(gmean[:, g * 8:(g + 1) * 8], hs[:],
                                 sel_sb[:, m % 15, :],
                                 start=(m % 15 == 0),
                                 stop=(m % 15 == 14) or (not do_pool and m == 1))
                # masked copy for max pool
                hm = dp.tile([128, F], dt.float32, tag="zr")
                nc.vector.tensor_scalar(hm[:], h[:], mask_sb[:, m:m + 1],
                                        negm_sb[:, m:m + 1],
                                        op0=mybir.AluOpType.mult,
                                        op1=mybir.AluOpType.add)
                htp = pp.tile([F, 128], dt.float32, tag="ztp")
                nc.tensor.transpose(htp[:], hm[:], id_sb[:])
                nc.scalar.copy(h3maxT[:, ts], htp[:])
            nc.vector.tensor_reduce(
                combT[0:F, :], h3maxT[:].rearrange("p (g s) -> p g s", s=SLOT),
                axis=mybir.AxisListType.X, op=mybir.AluOpType.max)
            nc.scalar.copy(combT[F:128, :], gmean[:])
            nc.vector.tensor_mul(combT[F:128, :], combT[F:128, :], cntinv_sb[F:128, :])
            logits = pp.tile([GPC, 2], dt.float32, tag="logits")
            nc.tensor.matmul(logits[:], combT[:], wc_sb[:], start=True, stop=True)
            lsb = dp.tile([GPC, 2], dt.float32, tag="lsb")
            nc.scalar.copy(lsb[:], logits[:])
            nc.vector.tensor_add(lsb[:], lsb[:], bc_sb[:])
            mx = dp.tile([GPC, 1], dt.float32, tag="mx")
            nc.vector.tensor_reduce(mx[:], lsb[:], axis=mybir.AxisListType.X,
                                    op=mybir.AluOpType.max)
            nmx = dp.tile([GPC, 1], dt.float32, tag="nmx")
            nc.scalar.mul(nmx[:], mx[:], -1.0)
            e = dp.tile([GPC, 2], dt.float32, tag="e")
            nc.scalar.activation(e[:], lsb[:], AF.Exp, bias=nmx[:])
            s = dp.tile([GPC, 1], dt.float32, tag="s")
            nc.vector.tensor_reduce(s[:], e[:], axis=mybir.AxisListType.X,
                                    op=mybir.AluOpType.add)
            r = dp.tile([GPC, 1], dt.float32, tag="r")
            nc.vector.reciprocal(r[:], s[:])
            o = dp.tile([GPC, 2], dt.float32, tag="o")
            nc.vector.tensor_scalar_mul(o[:], e[:], r[:])
            nc.sync.dma_start(t_out.ap(), o[:])
    nc.compile()
    return nc


def _prep(x, W1, b1, W2, b2, W3, b3, Wc, bc, edge_index, batch):
    N = x.shape[0]
    E = edge_index.shape[1]
    G = int(batch.max()) + 1
    assert G % NC == 0, G
    GPC = G // NC
    src = np.asarray(edge_index[0], dtype=np.int64)
    dst = np.asarray(edge_index[1], dtype=np.int64)
    batch = np.asarray(batch, dtype=np.int64)
    assert np.all(np.diff(batch) >= 0), "batch must be sorted"

    deg = np.bincount(dst, minlength=N).astype(np.float64) + 1.0
    dinv = (1.0 / np.sqrt(deg)).astype(np.float32)

    gstart = np.searchsorted(batch, np.arange(G))
    gcnt = np.diff(np.append(gstart, N))
    assert gcnt.min() >= 1, "empty graph"
    SLOT = 240
    assert gcnt.max() <= SLOT, f"graph too large: {gcnt.max()}"
    R = GPC * SLOT                      # 15360 rows per core
    assert R % 128 == 0 and 2 * R < 32768
    ntiles = R // 128
    assert ntiles % 15 == 0
    TRASH = R

    nodes = np.arange(N)
    rank_in_graph = nodes - gstart[batch]
    row_local = (batch % GPC) * SLOT + rank_in_graph      # row within owner core
    owner = batch // GPC

    # ---- per-core edge streams ----
    e_owner = owner[dst]
    e_bkt = (owner[src] >> 1).astype(np.int64)
    e_dloc = row_local[dst]
    e_gloc = (owner[src] - 2 * e_bkt) * R + row_local[src]  # idx within bucket table

    per_core = []
    max_rank = 0
    for c in range(NC):
        m = e_owner == c
        b, d, g = e_bkt[m], e_dloc[m], e_gloc[m]
        order = np.lexsort((g, d, b))
        b, d, g = b[order], d[order], g[order]
        key = b * (R + 1) + d
        new = np.r_[True, key[1:] != key[:-1]] if len(key) else np.array([], bool)
        idx = np.arange(len(key))
        grp_start = np.maximum.accumulate(np.where(new, idx, 0)) if len(key) else idx
        rank = idx - grp_start
        order2 = np.lexsort((d, rank, b))
        b, d, g, rank = b[order2], d[order2], g[order2], rank[order2]
        per_core.append((b, d, g, rank))
        if len(rank):
            max_rank = max(max_rank, int(rank.max()))

    # global segment sizes: (bucket, round) -> padded max count over cores
    nseg = {}
    for c in range(NC):
        b, d, g, rank = per_core[c]
        if len(b) == 0:
            continue
        segkey = b * (max_rank + 1) + rank
        uk, cnts = np.unique(segkey, return_counts=True)
        for k, cn in zip(uk, cnts):
            nseg[int(k)] = max(nseg.get(int(k), 0), int(cn))
    seg_keys = sorted(nseg)
    CAP = 1024          # max tokens per SWDGE gather/scatter instruction
    seg_pad = {}
    segments = []       # (bucket, n) chunks, rounds split to <= CAP tokens
    for k in seg_keys:
        n = ((nseg[k] + 127) // 128) * 128
        seg_pad[k] = n
        bkt = k // (max_rank + 1)
        while n > 0:
            c = min(n, CAP)
            segments.append((bkt, c))
            n -= c
    TOT = sum(n for _, n in segments)

    # ---- per-core packed token arrays ----
    gidx_all, sidx_all = [], []
    for c in range(NC):
        b, d, g, rank = per_core[c]
        segkey = b * (max_rank + 1) + rank
        gi = np.zeros(TOT, np.int64)
        si = np.full(TOT, TRASH, np.int64)
        off = 0
        # tokens are already sorted by (b, rank, d) == sorted segkey
        starts = np.searchsorted(segkey, np.array(seg_keys))
        ends = np.searchsorted(segkey, np.array(seg_keys), side="right")
        for k in range(len(seg_keys)):
            s0, e0 = starts[k], ends[k]
            cnt = e0 - s0
            gi[off:off + cnt] = g[s0:e0]
            si[off:off + cnt] = d[s0:e0]
            off += seg_pad[seg_keys[k]]
        gidx_all.append(_pack_idx(gi))
        sidx_all.append(_pack_idx(si))

    # ---- dense-phase per-core data ----
    W1p = np.zeros((4, F), np.float32); W1p[:3, :W1.shape[1]] = W1
    W2p = np.zeros((F, F), np.float32); W2p[:W2.shape[0], :] = W2
    W3p = np.asarray(W3, np.float32)
    Wcp = np.asarray(Wc, np.float32)
    b1p = np.zeros(F, np.float32); b1p[:b1.shape[0]] = b1
    b2p = np.asarray(b2, np.float32)
    b3p = np.asarray(b3, np.float32)
    ident = np.eye(128, dtype=np.float32)

    in_maps = []
    for c in range(NC):
        sel_nodes = nodes[owner == c]
        rl = row_local[sel_nodes]
        xT = np.zeros((4, R), np.float32)
        xT[:3, rl] = np.asarray(x, np.float32)[sel_nodes].T
        dv = np.zeros(R, np.float32)
        dv[rl] = dinv[sel_nodes]
        valid = np.zeros(R, np.float32)
        valid[rl] = 1.0
        cnt_c = gcnt[c * GPC:(c + 1) * GPC].astype(np.float32)
        sel = np.zeros((128, 15, 8), np.float32)
        rows = (np.arange(15 * 128)).reshape(15, 128)
        slot_in_grp = rows // SLOT
        for j in range(15):
            sel[np.arange(128), j, slot_in_grp[j]] = 1.0
        cntinv = np.tile((1.0 / cnt_c)[None, :], (128, 1)).astype(np.float32)
        mask = valid
        negm = ((1.0 - valid) * NEG_BIG).astype(np.float32)
        in_maps.append({
            "xT": xT,
            "gidx": gidx_all[c],
            "sidx": sidx_all[c],
            "dinv": dv.reshape(ntiles, 128).T.copy(),
            "mask": mask.reshape(ntiles, 128).T.copy(),
            "negm": negm.reshape(ntiles, 128).T.copy(),
            "sel": sel,
            "cntinv": cntinv,
            "w1": W1p, "w2": W2p, "w3": W3p, "wc": Wcp,
            "b1r": np.tile(b1p, (128, 1)),
            "b2r": np.tile(b2p, (128, 1)),
            "b3r": np.tile(b3p, (128, 1)),
            "bcr": np.tile(np.asarray(bc, np.float32), (GPC, 1)),
            "ident": ident,
        })
    cfg = dict(R=R, ntiles=ntiles, segments=segments, GPC=GPC, SLOT=SLOT,
               TOT=TOT)
    return in_maps, cfg


def kernel(x, W1, b1, W2, b2, W3, b3, Wc, bc, edge_index, batch):
    in_maps, cfg = _prep(x, W1, b1, W2, b2, W3, b3, Wc, bc, edge_index, batch)
    GPC, SLOT = cfg["GPC"], cfg["SLOT"]
    nc = _build_nc(cfg["R"], cfg["ntiles"], cfg["segments"], GPC, SLOT, cfg["TOT"])
    import os as _os
    _trace = _os.environ.get("GCN_TRACE", "0") == "1"
    res = run_bass_kernel_spmd(nc, in_maps, list(range(NC)), trace=_trace)
    global LAST_EXEC_NS, LAST_RESULT
    LAST_EXEC_NS = res.exec_time_ns
    LAST_RESULT = res
    outs = []
    for c in range(NC):
        o = res.results[c]["out"][:GPC].astype(np.float32)
        outs.append(o)
    return np.concatenate(outs, axis=0)



# revision 5
# speedup vs baseline: 2.6653x; 2.6653x over previous
"""GCN (3-layer) + graph pooling kernel for 8 Trainium2 NeuronCores.

Sharding: each core owns 64 of the 512 graphs (contiguous node range, since
`batch` is sorted). Every graph gets a fixed SLOT=240-row slot so per-core
layouts are uniform (SPMD). Edges are assigned to the core owning their dst.

Edge phase: per-edge dma_gather (256B rows) from the AllGather'd table on 4
rotating SWDGE queues (ring-drain parallelism ~2.3ns/token), aggregated
on-chip: edges sorted by (super-phase, src-bucket, dst-window); per 128-token
tile a one-hot S matrix (DVE iota==dpos compare) routes messages into a PSUM
window accumulator via TensorE matmul. No dma_scatter_add at all. Self-loop
terms are added from the local bounce buffer during the PSUM flush; the agg
table lives in SBUF (bf16) for the next dense phase.
"""
import numpy as np

import concourse.bacc as bacc
import concourse.bass as bass
import concourse.mybir as mybir
import concourse.tile as tile
from concourse.bass_utils import run_bass_kernel_spmd

LAST_EXEC_NS = None
LAST_RESULT = None

F = 64          # padded feature width (layer1's 32 is zero-padded to 64)
NC = 8          # cores
NEG_BIG = -1.0e30
WPS = 15        # dst windows (128 rows) per super-phase
NSP = 8         # super-phases (WPS*NSP*128 == R)
CAP = 1024      # max tokens per gather instruction
NQ = 4          # SWDGE queues


def _pack_idx(flat: np.ndarray) -> np.ndarray:
    """token i -> [i % 16, i // 16], replicated down to 128 partitions."""
    t = len(flat)
    a = np.ascontiguousarray(flat.astype(np.int16).reshape(t // 16, 16).T)
    return np.tile(a, (8, 1))


def _build_nc(R, ntiles, sched, GPC, SLOT, TOT, NTILE):
    nc = bacc.Bacc("TRN2", target_bir_lowering=False, debug=False,
                   num_devices=NC, dynamic_dma_scratch_size=65536,
                   num_swdge_queues=NQ)
    dt = mybir.dt
    AF = mybir.ActivationFunctionType

    # ---- external inputs ----
    t_xnm = nc.dram_tensor("xnm", [128, ntiles, 4], dt.float32, kind="ExternalInput")
    t_gidx = nc.dram_tensor("gidx", [128, TOT // 16], dt.int16, kind="ExternalInput")
    t_dpos = nc.dram_tensor("dpos", [128, NTILE], dt.float32, kind="ExternalInput")
    t_dinv = nc.dram_tensor("dinv", [128, ntiles], dt.float32, kind="ExternalInput")
    t_mask = nc.dram_tensor("mask", [128, ntiles], dt.float32, kind="ExternalInput")
    t_negm = nc.dram_tensor("negm", [128, ntiles], dt.float32, kind="ExternalInput")
    t_sel = nc.dram_tensor("sel", [128, 15, 8], dt.float32, kind="ExternalInput")
    t_cntinv = nc.dram_tensor("cntinv", [128, GPC], dt.float32, kind="ExternalInput")
    t_iota = nc.dram_tensor("iota", [128, 128], dt.float32, kind="ExternalInput")
    t_w1 = nc.dram_tensor("w1", [4, F], dt.float32, kind="ExternalInput")
    t_w2 = nc.dram_tensor("w2", [F, F], dt.float32, kind="ExternalInput")
    t_w3 = nc.dram_tensor("w3", [F, F], dt.float32, kind="ExternalInput")
    t_wc = nc.dram_tensor("wc", [128, 2], dt.float32, kind="ExternalInput")
    t_b1 = nc.dram_tensor("b1r", [128, F], dt.float32, kind="ExternalInput")
    t_b2 = nc.dram_tensor("b2r", [128, F], dt.float32, kind="ExternalInput")
    t_b3 = nc.dram_tensor("b3r", [128, F], dt.float32, kind="ExternalInput")
    t_bc = nc.dram_tensor("bcr", [GPC, 2], dt.float32, kind="ExternalInput")
    t_id = nc.dram_tensor("ident", [128, 128], dt.float32, kind="ExternalInput")
    t_out = nc.dram_tensor("out", [GPC, 2], dt.float32, kind="ExternalOutput")

    # ---- internal DRAM ----
    t_bounce = [nc.dram_tensor(f"bounce{l}", [R, F], dt.float32) for l in range(3)]
    t_pfull = [nc.dram_tensor(f"pfull{l}", [NC * R, F], dt.float32,
                              addr_space="Shared") for l in range(3)]

    with tile.TileContext(nc) as tc:
        with (
            tc.tile_pool(name="persist", bufs=1) as pc,
            tc.tile_pool(name="dense", bufs=3) as dp,
            tc.tile_pool(name="idxp", bufs=2) as ip,
            tc.tile_pool(name="flush", bufs=2) as fl,
            tc.tile_pool(name="gat", bufs=6) as gp,
            tc.tile_pool(name="msg", bufs=6) as mp,
            tc.tile_pool(name="sgen", bufs=6) as sp_,
            tc.tile_pool(name="psum", bufs=2, space="PSUM") as pp,
            tc.tile_pool(name="psagg", bufs=1, space="PSUM") as pa,
        ):
            # ---- persistent tiles ----
            xnm_sb = pc.tile([128, ntiles, 4], dt.float32, tag="xnm")
            dinv_sb = pc.tile([128, ntiles], dt.float32, tag="dinv")
            mask_sb = pc.tile([128, ntiles], dt.float32, tag="mask")
            negm_sb = pc.tile([128, ntiles], dt.float32, tag="negm")
            sel_sb = pc.tile([128, 15, 8], dt.float32, tag="sel")
            cntinv_sb = pc.tile([128, GPC], dt.float32, tag="cntinv")
            iota_sb = pc.tile([128, 128], dt.float32, tag="iota")
            dpos_sb = pc.tile([128, NTILE], dt.float32, tag="dpos")
            w1_sb = pc.tile([4, F], dt.float32, tag="w1")
            w2_sb = pc.tile([F, F], dt.float32, tag="w2")
            w3_sb = pc.tile([F, F], dt.float32, tag="w3")
            wc_sb = pc.tile([128, 2], dt.float32, tag="wc")
            b1_sb = pc.tile([128, F], dt.float32, tag="b1")
            b2_sb = pc.tile([128, F], dt.float32, tag="b2")
            b3_sb = pc.tile([128, F], dt.float32, tag="b3")
            bc_sb = pc.tile([GPC, 2], dt.float32, tag="bc")
            id_sb = pc.tile([128, 128], dt.float32, tag="ident")
            agg_sb = pc.tile([128, ntiles, F], dt.bfloat16, tag="agg")
            h3maxT = pc.tile([F, R], dt.float32, tag="h3maxT")
            combT = pc.tile([128, GPC], dt.float32, tag="combT")

            nc.sync.dma_start(xnm_sb[:], t_xnm.ap())
            nc.sync.dma_start(dinv_sb[:], t_dinv.ap())
            nc.sync.dma_start(mask_sb[:], t_mask.ap())
            nc.sync.dma_start(negm_sb[:], t_negm.ap())
            nc.sync.dma_start(sel_sb[:], t_sel.ap())
            nc.sync.dma_start(cntinv_sb[:], t_cntinv.ap())
            nc.sync.dma_start(iota_sb[:], t_iota.ap())
            nc.sync.dma_start(dpos_sb[:], t_dpos.ap())
            nc.sync.dma_start(w1_sb[:], t_w1.ap())
            nc.sync.dma_start(w2_sb[:], t_w2.ap())
            nc.sync.dma_start(w3_sb[:], t_w3.ap())
            nc.sync.dma_start(wc_sb[:], t_wc.ap())
            nc.sync.dma_start(b1_sb[:], t_b1.ap())
            nc.sync.dma_start(b2_sb[:], t_b2.ap())
            nc.sync.dma_start(b3_sb[:], t_b3.ap())
            nc.sync.dma_start(bc_sb[:], t_bc.ap())
            nc.sync.dma_start(id_sb[:], t_id.ap())

            w_for = {1: w2_sb, 2: w3_sb}
            b_for = {1: b1_sb, 2: b2_sb}
            qctr = [0]

            # ---------------- dense phase of layer l (produce P_l)
            def dense_layer(l):
                for m in range(ntiles):
                    ts = slice(m * 128, (m + 1) * 128)
                    dcol = dinv_sb[:, m:m + 1]
                    if l == 0:
                        xtp = pp.tile([F, 128], dt.float32, tag="ztp")
                        nc.tensor.transpose(xtp[0:4, :], xnm_sb[:, m, :], id_sb[:])
                        xts = dp.tile([F, 128], dt.float32, tag="zts")
                        nc.scalar.copy(xts[0:4, :], xtp[0:4, :])
                        mm = pp.tile([128, F], dt.float32, tag="mm")
                        nc.tensor.matmul(mm[:], xts[0:3, :], w1_sb[0:3, :],
                                         start=True, stop=True)
                    else:
                        z = dp.tile([128, F], dt.float32, tag="z")
                        nc.vector.tensor_scalar_mul(z[:], agg_sb[:, m, :], dcol)
                        nc.vector.tensor_add(z[:], z[:], b_for[l][:])
                        zr = dp.tile([128, F], dt.float32, tag="zr")
                        nc.scalar.activation(zr[:], z[:], AF.Relu)
                        ztp = pp.tile([F, 128], dt.float32, tag="ztp")
                        nc.tensor.transpose(ztp[:], zr[:], id_sb[:])
                        zts = dp.tile([F, 128], dt.float32, tag="zts")
                        nc.scalar.copy(zts[:], ztp[:])
                        mm = pp.tile([128, F], dt.float32, tag="mm")
                        nc.tensor.matmul(mm[:], zts[:], w_for[l][:],
                                         start=True, stop=True)
                    ps = dp.tile([128, F], dt.float32, tag="ps")
                    nc.scalar.activation(ps[:], mm[:], AF.Copy, scale=dcol)
                    nc.sync.dma_start(t_bounce[l].ap()[ts], ps[:])
                nc.gpsimd.collective_compute(
                    "AllGather", mybir.AluOpType.bypass,
                    replica_groups=[list(range(NC))],
                    ins=[t_bounce[l].ap().opt()],
                    outs=[t_pfull[l].ap().opt()],
                )

            # ---------------- edge phase of layer l
            def edge_layer(l):
                for sp in range(NSP):
                    spd = sched[sp]
                    psw = pa.tile([128, WPS, F], dt.float32, tag="psw")
                    idx_sb = ip.tile([128, spd["ntok"] // 16], dt.int16, tag="gi")
                    o16 = spd["tok_off"] // 16
                    nc.sync.dma_start(idx_sb[:],
                                      t_gidx.ap()[:, o16:o16 + spd["ntok"] // 16])
                    for b in range(4):
                        src = t_pfull[l].ap()[b * 2 * R:(b + 1) * 2 * R]
                        gtbs = []
                        for (io, n) in spd["insts"][b]:
                            gt = gp.tile([128, CAP // 128, F], dt.float32, tag="gt")
                            nc.gpsimd.dma_gather(
                                gt[:, :n // 128, :], src,
                                idx_sb[:, io // 16:io // 16 + n // 16], n, n, F,
                                queue_num=qctr[0] % NQ)
                            qctr[0] += 1
                            gtb = mp.tile([128, CAP // 128, F], dt.bfloat16, tag="gtb")
                            nc.scalar.copy(gtb[:, :n // 128, :], gt[:, :n // 128, :])
                            gtbs.append(gtb)
                        for (w, col, ii, ofs, first, last) in spd["tiles"][b]:
                            S = sp_.tile([128, 128], dt.bfloat16, tag="S")
                            nc.vector.tensor_scalar(
                                S[:], iota_sb[:], dpos_sb[:, col:col + 1], None,
                                op0=mybir.AluOpType.is_equal)
                            nc.tensor.matmul(psw[:, w - sp * WPS, :], S[:],
                                             gtbs[ii][:, ofs, :],
                                             start=first, stop=last)
                    # flush: agg = psw + self-loop rows from bounce
                    selfsb = fl.tile([128, WPS, F], dt.float32, tag="selfsb")
                    rs = sp * WPS * 128
                    nc.sync.dma_start(
                        selfsb[:],
                        t_bounce[l].ap()[rs:rs + WPS * 128].rearrange(
                            "(t p) f -> p t f", p=128))
                    nc.vector.tensor_add(agg_sb[:, sp * WPS:(sp + 1) * WPS, :],
                                         psw[:], selfsb[:])

            for l in range(3):
                dense_layer(l)
                edge_layer(l)

            # ---------------- pooling + classifier ----------------
            gmean = pa.tile([F, GPC], dt.float32, tag="gmean")
            for m in range(ntiles):
                ts = slice(m * 128, (m + 1) * 128)
                h = dp.tile([128, F], dt.float32, tag="z")
                nc.vector.tensor_scalar_mul(h[:], agg_sb[:, m, :],
                                            dinv_sb[:, m:m + 1])
                nc.vector.tensor_add(h[:], h[:], b3_sb[:])
                hs = dp.tile([128, F], dt.float32, tag="hs")
                nc.vector.tensor_scalar_mul(hs[:], h[:], mask_sb[:, m:m + 1])
                g = m // 15
                nc.tensor.matmul(gmean[:, g * 8:(g + 1) * 8], hs[:],
                                 sel_sb[:, m % 15, :],
                                 start=(m % 15 == 0), stop=(m % 15 == 14))
                hm = dp.tile([128, F], dt.float32, tag="zr")
                nc.vector.tensor_scalar(hm[:], h[:], mask_sb[:, m:m + 1],
                                        negm_sb[:, m:m + 1],
                                        op0=mybir.AluOpType.mult,
                                        op1=mybir.AluOpType.add)
                htp = pp.tile([F, 128], dt.float32, tag="ztp")
                nc.tensor.transpose(htp[:], hm[:], id_sb[:])
                nc.scalar.copy(h3maxT[:, ts], htp[:])
            nc.vector.tensor_reduce(
                combT[0:F, :], h3maxT[:].rearrange("p (g s) -> p g s", s=SLOT),
                axis=mybir.AxisListType.X, op=mybir.AluOpType.max)
            nc.scalar.copy(combT[F:128, :], gmean[:])
            nc.vector.tensor_mul(combT[F:128, :], combT[F:128, :], cntinv_sb[F:128, :])
            logits = pa.tile([GPC, 2], dt.float32, tag="logits")
            nc.tensor.matmul(logits[:], combT[:], wc_sb[:], start=True, stop=True)
            lsb = dp.tile([GPC, 2], dt.float32, tag="lsb")
            nc.scalar.copy(lsb[:], logits[:])
            nc.vector.tensor_add(lsb[:], lsb[:], bc_sb[:])
            mx = dp.tile([GPC, 1], dt.float32, tag="mx")
            nc.vector.tensor_reduce(mx[:], lsb[:], axis=mybir.AxisListType.X,
                                    op=mybir.AluOpType.max)
            nmx = dp.tile([GPC, 1], dt.float32, tag="nmx")
            nc.scalar.mul(nmx[:], mx[:], -1.0)
            e = dp.tile([GPC, 2], dt.float32, tag="e")
            nc.scalar.activation(e[:], lsb[:], AF.Exp, bias=nmx[:])
            s = dp.tile([GPC, 1], dt.float32, tag="s")
            nc.vector.tensor_reduce(s[:], e[:], axis=mybir.AxisListType.X,
                                    op=mybir.AluOpType.add)
            r = dp.tile([GPC, 1], dt.float32, tag="r")
            nc.vector.reciprocal(r[:], s[:])
            o = dp.tile([GPC, 2], dt.float32, tag="o")
            nc.vector.tensor_scalar_mul(o[:], e[:], r[:])
            nc.sync.dma_start(t_out.ap(), o[:])
    nc.compile()
    return nc


def _prep(x, W1, b1, W2, b2, W3, b3, Wc, bc, edge_index, batch):
    N = x.shape[0]
    G = int(batch.max()) + 1
    assert G % NC == 0, G
    GPC = G // NC
    src = np.asarray(edge_index[0], dtype=np.int64)
    dst = np.asarray(edge_index[1], dtype=np.int64)
    batch = np.asarray(batch, dtype=np.int64)
    assert np.all(np.diff(batch) >= 0), "batch must be sorted"

    deg = np.bincount(dst, minlength=N).astype(np.float64) + 1.0
    dinv = (1.0 / np.sqrt(deg)).astype(np.float32)

    gstart = np.searchsorted(batch, np.arange(G))
    gcnt = np.diff(np.append(gstart, N))
    assert gcnt.min() >= 1, "empty graph"
    SLOT = 240
    assert gcnt.max() <= SLOT, f"graph too large: {gcnt.max()}"
    R = GPC * SLOT                      # 15360 rows per core
    assert R % 128 == 0 and 2 * R < 32768
    ntiles = R // 128
    assert ntiles == WPS * NSP and ntiles % 15 == 0

    nodes = np.arange(N)
    rank_in_graph = nodes - gstart[batch]
    row_local = (batch % GPC) * SLOT + rank_in_graph      # row within owner core
    owner = batch // GPC

    # ---- per-core edge streams: sorted by (sp, bucket, window, src-row) ----
    e_owner = owner[dst]
    e_bkt = (owner[src] >> 1).astype(np.int64)
    e_dloc = row_local[dst]
    e_gloc = (owner[src] & 1) * R + row_local[src]
    e_w = e_dloc >> 7                     # dst window 0..ntiles-1

    # per-core counts per (sp, b, w)
    counts = np.zeros((NC, NSP, 4, WPS), np.int64)
    per_core = []
    for c in range(NC):
        m = e_owner == c
        b, w, dl, g = e_bkt[m], e_w[m], e_dloc[m], e_gloc[m]
        spv = w // WPS
        order = np.lexsort((g, w, b, spv))
        b, w, dl, g = b[order], w[order], dl[order], g[order]
        np.add.at(counts[c], (spv[order], b, w % WPS), 1)
        per_core.append((b, w, dl, g))

    cnt_max = counts.max(axis=0)                       # [NSP, 4, WPS]
    n_pad = ((cnt_max + 127) // 128) * 128
    n_pad = np.maximum(n_pad, 128)

    # schedule (uniform across cores)
    sched = []
    tok_off = 0
    col = 0
    for sp in range(NSP):
        spd = {"tok_off": tok_off, "insts": [], "tiles": []}
        ntok_sp = 0
        for b in range(4):
            io = ntok_sp                                # token offset within sp
            insts = []
            tiles = []
            # split this bucket's stream into <=CAP instructions
            total_b = int(n_pad[sp, b].sum())
            pos = 0
            while pos < total_b:
                n = min(CAP, total_b - pos)
                insts.append((io + pos, n))
                pos += n
            # tiles in stream order with window labels
            tpos = 0
            first_seen = set()
            for w in range(WPS):
                nt = int(n_pad[sp, b, w]) // 128
                for t in range(nt):
                    ii = tpos // CAP                    # instruction index
                    ofs = (tpos % CAP) // 128
                    first = (b == 0 and t == 0)
                    last = (b == 3 and t == nt - 1)
                    tiles.append((sp * WPS + w, col, ii, ofs, first, last))
                    col += 1
                    tpos += 128
            spd["insts"].append(insts)
            spd["tiles"].append(tiles)
            ntok_sp += total_b
        spd["ntok"] = ntok_sp
        tok_off += ntok_sp
        sched.append(spd)
    TOT = tok_off
    NTILE = col

    # per-core packed gidx + dpos
    gidx_all, dpos_all = [], []
    for c in range(NC):
        b, w, dl, g = per_core[c]
        spv = w // WPS
        wloc = w % WPS
        gi = np.zeros(TOT, np.int64)
        dp_ = np.full(NTILE * 128, -1.0, np.float32)
        # walk in the same (sp, b, w) order
        starts = {}
        o = 0
        for sp in range(NSP):
            for bb in range(4):
                for ww in range(WPS):
                    starts[(sp, bb, ww)] = o
                    o += int(n_pad[sp, bb, ww])
        tile_off = {}
        o = 0
        for sp in range(NSP):
            for bb in range(4):
                for ww in range(WPS):
                    tile_off[(sp, bb, ww)] = o
                    o += int(n_pad[sp, bb, ww])
        key = spv * (4 * WPS) + b * WPS + wloc
        order = np.argsort(key, kind="stable")   # already sorted; stable keeps g order
        ks = key[order]
        # positions within each (sp,b,w) run
        runstart = np.r_[0, np.flatnonzero(np.diff(ks)) + 1]
        runkey = ks[runstart]
        runlen = np.diff(np.append(runstart, len(ks)))
        for rk, rs, rl in zip(runkey, runstart, runlen):
            sp, rem = divmod(int(rk), 4 * WPS)
            bb, ww = divmod(rem, WPS)
            base = starts[(sp, bb, ww)]
            idxs = order[rs:rs + rl]
            gi[base:base + rl] = g[idxs]
            dp_[base:base + rl] = (dl[idxs] & 127).astype(np.float32)
        gidx_all.append(_pack_idx(gi))
        # dpos: token position base == tile column base * 128 (same ordering)
        dpos_all.append(np.ascontiguousarray(
            dp_.reshape(NTILE, 128).T))

    # ---- dense-phase per-core data ----
    W1p = np.zeros((4, F), np.float32); W1p[:3, :W1.shape[1]] = W1
    W2p = np.zeros((F, F), np.float32); W2p[:W2.shape[0], :] = W2
    W3p = np.asarray(W3, np.float32)
    Wcp = np.asarray(Wc, np.float32)
    b1p = np.zeros(F, np.float32); b1p[:b1.shape[0]] = b1
    b2p = np.asarray(b2, np.float32)
    b3p = np.asarray(b3, np.float32)
    ident = np.eye(128, dtype=np.float32)
    iota = np.tile(np.arange(128, dtype=np.float32)[None, :], (128, 1))

    in_maps = []
    for c in range(NC):
        sel_nodes = nodes[owner == c]
        rl = row_local[sel_nodes]
        xnm = np.zeros((R, 4), np.float32)
        xnm[rl, :3] = np.asarray(x, np.float32)[sel_nodes]
        dv = np.zeros(R, np.float32)
        dv[rl] = dinv[sel_nodes]
        valid = np.zeros(R, np.float32)
        valid[rl] = 1.0
        cnt_c = gcnt[c * GPC:(c + 1) * GPC].astype(np.float32)
        sel = np.zeros((128, 15, 8), np.float32)
        rows = (np.arange(15 * 128)).reshape(15, 128)
        slot_in_grp = rows // SLOT
        for j in range(15):
            sel[np.arange(128), j, slot_in_grp[j]] = 1.0
        cntinv = np.tile((1.0 / cnt_c)[None, :], (128, 1)).astype(np.float32)
        mask = valid
        negm = ((1.0 - valid) * NEG_BIG).astype(np.float32)
        in_maps.append({
            "xnm": xnm.reshape(ntiles, 128, 4).transpose(1, 0, 2).copy(),
            "gidx": gidx_all[c],
            "dpos": dpos_all[c],
            "dinv": dv.reshape(ntiles, 128).T.copy(),
            "mask": mask.reshape(ntiles, 128).T.copy(),
            "negm": negm.reshape(ntiles, 128).T.copy(),
            "sel": sel,
            "cntinv": cntinv,
            "iota": iota,
            "w1": W1p, "w2": W2p, "w3": W3p, "wc": Wcp,
            "b1r": np.tile(b1p, (128, 1)),
            "b2r": np.tile(b2p, (128, 1)),
            "b3r": np.tile(b3p, (128, 1)),
            "bcr": np.tile(np.asarray(bc, np.float32), (GPC, 1)),
            "ident": ident,
        })
    cfg = dict(R=R, ntiles=ntiles, sched=sched, GPC=GPC, SLOT=SLOT, TOT=TOT,
               NTILE=NTILE)
    return in_maps, cfg


def kernel(x, W1, b1, W2, b2, W3, b3, Wc, bc, edge_index, batch):
    in_maps, cfg = _prep(x, W1, b1, W2, b2, W3, b3, Wc, bc, edge_index, batch)
    GPC, SLOT = cfg["GPC"], cfg["SLOT"]
    nc = _build_nc(cfg["R"], cfg["ntiles"], cfg["sched"], GPC, SLOT,
                   cfg["TOT"], cfg["NTILE"])
    import os as _os
    _trace = _os.environ.get("GCN_TRACE", "0") == "1"
    res = run_bass_kernel_spmd(nc, in_maps, list(range(NC)), trace=_trace)
    global LAST_EXEC_NS, LAST_RESULT
    LAST_EXEC_NS = res.exec_time_ns
    LAST_RESULT = res
    outs = []
    for c in range(NC):
        o = res.results[c]["out"][:GPC].astype(np.float32)
        outs.append(o)
    return np.concatenate(outs, axis=0)


# revision 9
# speedup vs baseline: 2.8041x; 1.0521x over previous
"""GCN (3-layer) + graph pooling kernel for 8 Trainium2 NeuronCores.

Sharding: each core owns 64 of the 512 graphs (contiguous node range, since
`batch` is sorted). Every graph gets a fixed SLOT=240-row slot so per-core
layouts are uniform (SPMD). Edges are assigned to the core owning their dst.

Edge phase: per-edge dma_gather (256B rows) from the AllGather'd table on 4
rotating SWDGE queues (ring-drain parallelism ~2.3ns/token), aggregated
on-chip: edges sorted by (super-phase, src-bucket, dst-window); per 128-token
tile a one-hot S matrix (DVE iota==dpos compare) routes messages into a PSUM
window accumulator via TensorE matmul. No dma_scatter_add at all. Self-loop
terms are added from the local bounce buffer during the PSUM flush; the agg
table lives in SBUF (bf16) for the next dense phase.
"""
import numpy as np

import concourse.bacc as bacc
import concourse.bass as bass
import concourse.mybir as mybir
import concourse.tile as tile
from concourse.bass_utils import run_bass_kernel_spmd

LAST_EXEC_NS = None
LAST_RESULT = None

F = 64          # padded feature width (layer1's 32 is zero-padded to 64)
NC = 8          # cores
NEG_BIG = -1.0e30
WPS = 15        # dst windows (128 rows) per super-phase
NSP = 8         # super-phases (WPS*NSP*128 == R)
CAP = 1024      # max tokens per gather instruction
NQ = 4          # SWDGE queues


def _pack_idx(flat: np.ndarray) -> np.ndarray:
    """token i -> [i % 16, i // 16], replicated down to 128 partitions."""
    t = len(flat)
    a = np.ascontiguousarray(flat.astype(np.int16).reshape(t // 16, 16).T)
    return np.tile(a, (8, 1))


def _build_nc(R, ntiles, sched, GPC, SLOT, TOT, NTILE):
    nc = bacc.Bacc("TRN2", target_bir_lowering=False, debug=False,
                   num_devices=NC, dynamic_dma_scratch_size=65536,
                   num_swdge_queues=NQ)
    dt = mybir.dt
    AF = mybir.ActivationFunctionType

    # ---- external inputs ----
    t_xnm = nc.dram_tensor("xnm", [128, ntiles, 4], dt.float32, kind="ExternalInput")
    t_gidx = nc.dram_tensor("gidx", [128, TOT // 16], dt.int16, kind="ExternalInput")
    t_dpos = nc.dram_tensor("dpos", [128, NTILE], dt.float32, kind="ExternalInput")
    t_dinv = nc.dram_tensor("dinv", [128, ntiles], dt.float32, kind="ExternalInput")
    t_mask = nc.dram_tensor("mask", [128, ntiles], dt.float32, kind="ExternalInput")
    t_negm = nc.dram_tensor("negm", [128, ntiles], dt.float32, kind="ExternalInput")
    t_sel = nc.dram_tensor("sel", [128, 15, 8], dt.float32, kind="ExternalInput")
    t_cntinv = nc.dram_tensor("cntinv", [128, GPC], dt.float32, kind="ExternalInput")
    t_iota = nc.dram_tensor("iota", [128, 128], dt.float32, kind="ExternalInput")
    t_w1 = nc.dram_tensor("w1", [4, F], dt.bfloat16, kind="ExternalInput")
    t_w2 = nc.dram_tensor("w2", [F, F], dt.bfloat16, kind="ExternalInput")
    t_w3 = nc.dram_tensor("w3", [F, F], dt.bfloat16, kind="ExternalInput")
    t_wc = nc.dram_tensor("wc", [128, 2], dt.float32, kind="ExternalInput")
    t_b1 = nc.dram_tensor("b1r", [128, F], dt.float32, kind="ExternalInput")
    t_b2 = nc.dram_tensor("b2r", [128, F], dt.float32, kind="ExternalInput")
    t_b3 = nc.dram_tensor("b3r", [128, F], dt.float32, kind="ExternalInput")
    t_bc = nc.dram_tensor("bcr", [GPC, 2], dt.float32, kind="ExternalInput")
    t_id = nc.dram_tensor("ident", [128, 128], dt.float32, kind="ExternalInput")
    t_out = nc.dram_tensor("out", [GPC, 2], dt.float32, kind="ExternalOutput")

    # ---- internal DRAM ----
    t_bounce = [nc.dram_tensor(f"bounce{l}", [R, F], dt.float32) for l in range(3)]
    t_pfull = [nc.dram_tensor(f"pfull{l}", [NC * R, F], dt.float32,
                              addr_space="Shared") for l in range(3)]

    with tile.TileContext(nc) as tc:
        with (
            tc.tile_pool(name="persist", bufs=1) as pc,
            tc.tile_pool(name="dense", bufs=3) as dp,
            tc.tile_pool(name="idxp", bufs=2) as ip,
            tc.tile_pool(name="flush", bufs=2) as fl,
            tc.tile_pool(name="gat", bufs=10) as gp,
            tc.tile_pool(name="msg", bufs=10) as mp,
            tc.tile_pool(name="sgen", bufs=12) as sp_,
            tc.tile_pool(name="psum", bufs=2, space="PSUM") as pp,
            tc.tile_pool(name="psagg", bufs=1, space="PSUM") as pa,
        ):
            # ---- persistent tiles ----
            xnm_sb = pc.tile([128, ntiles, 4], dt.float32, tag="xnm")
            dinv_sb = pc.tile([128, ntiles], dt.float32, tag="dinv")
            mask_sb = pc.tile([128, ntiles], dt.float32, tag="mask")
            negm_sb = pc.tile([128, ntiles], dt.float32, tag="negm")
            sel_sb = pc.tile([128, 15, 8], dt.float32, tag="sel")
            cntinv_sb = pc.tile([128, GPC], dt.float32, tag="cntinv")
            iota_sb = pc.tile([128, 128], dt.float32, tag="iota")
            dpos_sb = pc.tile([128, NTILE], dt.float32, tag="dpos")
            w1_sb = pc.tile([4, F], dt.bfloat16, tag="w1")
            w2_sb = pc.tile([F, F], dt.bfloat16, tag="w2")
            w3_sb = pc.tile([F, F], dt.bfloat16, tag="w3")
            wc_sb = pc.tile([128, 2], dt.float32, tag="wc")
            b1_sb = pc.tile([128, F], dt.float32, tag="b1")
            b2_sb = pc.tile([128, F], dt.float32, tag="b2")
            b3_sb = pc.tile([128, F], dt.float32, tag="b3")
            bc_sb = pc.tile([GPC, 2], dt.float32, tag="bc")
            id_sb = pc.tile([128, 128], dt.float32, tag="ident")
            agg_sb = pc.tile([128, ntiles, F], dt.bfloat16, tag="agg")
            h3maxT = pc.tile([F, R], dt.float32, tag="h3maxT")
            combT = pc.tile([128, GPC], dt.float32, tag="combT")

            nc.sync.dma_start(xnm_sb[:], t_xnm.ap())
            nc.sync.dma_start(dinv_sb[:], t_dinv.ap())
            nc.sync.dma_start(mask_sb[:], t_mask.ap())
            nc.sync.dma_start(negm_sb[:], t_negm.ap())
            nc.sync.dma_start(sel_sb[:], t_sel.ap())
            nc.sync.dma_start(cntinv_sb[:], t_cntinv.ap())
            nc.sync.dma_start(iota_sb[:], t_iota.ap())
            nc.sync.dma_start(dpos_sb[:], t_dpos.ap())
            nc.sync.dma_start(w1_sb[:], t_w1.ap())
            nc.sync.dma_start(w2_sb[:], t_w2.ap())
            nc.sync.dma_start(w3_sb[:], t_w3.ap())
            nc.sync.dma_start(wc_sb[:], t_wc.ap())
            nc.sync.dma_start(b1_sb[:], t_b1.ap())
            nc.sync.dma_start(b2_sb[:], t_b2.ap())
            nc.sync.dma_start(b3_sb[:], t_b3.ap())
            nc.sync.dma_start(bc_sb[:], t_bc.ap())
            nc.sync.dma_start(id_sb[:], t_id.ap())

            w_for = {1: w2_sb, 2: w3_sb}
            b_for = {1: b1_sb, 2: b2_sb}
            qctr = [0]

            # ---- dense tile: compute ps row-tile m of layer l, write bounce[l]
            def dense_tile(l, m):
                ts = slice(m * 128, (m + 1) * 128)
                dcol = dinv_sb[:, m:m + 1]
                if l == 0:
                    xtp = pp.tile([F, 128], dt.float32, tag="ztp")
                    nc.tensor.transpose(xtp[0:4, :], xnm_sb[:, m, :], id_sb[:])
                    xts = dp.tile([F, 128], dt.bfloat16, tag="zts")
                    nc.scalar.copy(xts[0:4, :], xtp[0:4, :])
                    mm = pp.tile([128, F], dt.float32, tag="mm")
                    nc.tensor.matmul(mm[:], xts[0:3, :], w1_sb[0:3, :],
                                     start=True, stop=True)
                else:
                    z = dp.tile([128, F], dt.float32, tag="z")
                    nc.vector.tensor_scalar_mul(z[:], agg_sb[:, m, :], dcol)
                    nc.vector.tensor_add(z[:], z[:], b_for[l][:])
                    zr = dp.tile([128, F], dt.float32, tag="zr")
                    nc.scalar.activation(zr[:], z[:], AF.Relu)
                    ztp = pp.tile([F, 128], dt.float32, tag="ztp")
                    nc.tensor.transpose(ztp[:], zr[:], id_sb[:])
                    zts = dp.tile([F, 128], dt.bfloat16, tag="zts")
                    nc.scalar.copy(zts[:], ztp[:])
                    mm = pp.tile([128, F], dt.float32, tag="mm")
                    nc.tensor.matmul(mm[:], zts[:], w_for[l][:],
                                     start=True, stop=True)
                ps = dp.tile([128, F], dt.float32, tag="ps")
                nc.scalar.activation(ps[:], mm[:], AF.Copy, scale=dcol)
                nc.sync.dma_start(t_bounce[l].ap()[ts], ps[:])

            def allgather(l):
                nc.gpsimd.collective_compute(
                    "AllGather", mybir.AluOpType.bypass,
                    replica_groups=[list(range(NC))],
                    ins=[t_bounce[l].ap().opt()],
                    outs=[t_pfull[l].ap().opt()],
                )

            # ---- pooling tile (layer-2 output): gmean matmul + max-pool stage
            def pool_tile(m):
                ts = slice(m * 128, (m + 1) * 128)
                h = dp.tile([128, F], dt.float32, tag="z")
                nc.vector.tensor_scalar_mul(h[:], agg_sb[:, m, :],
                                            dinv_sb[:, m:m + 1])
                nc.vector.tensor_add(h[:], h[:], b3_sb[:])
                hs = dp.tile([128, F], dt.float32, tag="hs")
                nc.vector.tensor_scalar_mul(hs[:], h[:], mask_sb[:, m:m + 1])
                g = m // 15
                nc.tensor.matmul(gmean[:, g * 8:(g + 1) * 8], hs[:],
                                 sel_sb[:, m % 15, :],
                                 start=(m % 15 == 0), stop=(m % 15 == 14))
                hm = dp.tile([128, F], dt.float32, tag="zr")
                nc.vector.tensor_scalar(hm[:], h[:], mask_sb[:, m:m + 1],
                                        negm_sb[:, m:m + 1],
                                        op0=mybir.AluOpType.mult,
                                        op1=mybir.AluOpType.add)
                htp = pp.tile([F, 128], dt.float32, tag="ztp")
                nc.tensor.transpose(htp[:], hm[:], id_sb[:])
                nc.scalar.copy(h3maxT[:, ts], htp[:])

            # ---- edge phase of layer l; folds dense of l+1 (or pooling) per sp
            def edge_layer(l):
                for sp in range(NSP):
                    spd = sched[sp]
                    psw = pa.tile([128, WPS, F], dt.float32, tag="psw")
                    idx_sb = ip.tile([128, spd["ntok"] // 16], dt.int16, tag="gi")
                    o16 = spd["tok_off"] // 16
                    nc.sync.dma_start(idx_sb[:],
                                      t_gidx.ap()[:, o16:o16 + spd["ntok"] // 16])
                    for b in range(4):
                        src = t_pfull[l].ap()[b * 2 * R:(b + 1) * 2 * R]
                        gtbs = []
                        for (io, n) in spd["insts"][b]:
                            gt = gp.tile([128, CAP // 128, F], dt.float32, tag="gt")
                            nc.gpsimd.dma_gather(
                                gt[:, :n // 128, :], src,
                                idx_sb[:, io // 16:io // 16 + n // 16], n, n, F,
                                queue_num=qctr[0] % NQ)
                            qctr[0] += 1
                            gtb = mp.tile([128, CAP // 128, F], dt.bfloat16, tag="gtb")
                            nc.scalar.copy(gtb[:, :n // 128, :], gt[:, :n // 128, :])
                            gtbs.append(gtb)
                        for (w, col, ii, ofs, first, last) in spd["tiles"][b]:
                            S = sp_.tile([128, 128], dt.bfloat16, tag="S")
                            nc.vector.tensor_scalar(
                                S[:], iota_sb[:], dpos_sb[:, col:col + 1], None,
                                op0=mybir.AluOpType.is_equal)
                            nc.tensor.matmul(psw[:, w - sp * WPS, :], S[:],
                                             gtbs[ii][:, ofs, :],
                                             start=first, stop=last)
                    # flush: agg = psw + self-loop rows from bounce
                    selfsb = fl.tile([128, WPS, F], dt.float32, tag="selfsb")
                    rs = sp * WPS * 128
                    nc.sync.dma_start(
                        selfsb[:],
                        t_bounce[l].ap()[rs:rs + WPS * 128].rearrange(
                            "(t p) f -> p t f", p=128))
                    nc.vector.tensor_add(agg_sb[:, sp * WPS:(sp + 1) * WPS, :],
                                         psw[:], selfsb[:])
                    # fold next layer's dense (or pooling) for this sp's tiles
                    for m in range(sp * WPS, (sp + 1) * WPS):
                        if l < 2:
                            dense_tile(l + 1, m)
                        else:
                            pool_tile(m)

            gmean = pa.tile([F, GPC], dt.float32, tag="gmean")
            for m in range(ntiles):
                dense_tile(0, m)
            allgather(0)
            for l in range(3):
                edge_layer(l)
                if l < 2:
                    allgather(l + 1)

            # ---------------- pooling finalize + classifier ----------------
            nc.vector.tensor_reduce(
                combT[0:F, :], h3maxT[:].rearrange("p (g s) -> p g s", s=SLOT),
                axis=mybir.AxisListType.X, op=mybir.AluOpType.max)
            nc.scalar.copy(combT[F:128, :], gmean[:])
            nc.vector.tensor_mul(combT[F:128, :], combT[F:128, :], cntinv_sb[F:128, :])
            logits = pa.tile([GPC, 2], dt.float32, tag="logits")
            nc.tensor.matmul(logits[:], combT[:], wc_sb[:], start=True, stop=True)
            lsb = dp.tile([GPC, 2], dt.float32, tag="lsb")
            nc.scalar.copy(lsb[:], logits[:])
            nc.vector.tensor_add(lsb[:], lsb[:], bc_sb[:])
            mx = dp.tile([GPC, 1], dt.float32, tag="mx")
            nc.vector.tensor_reduce(mx[:], lsb[:], axis=mybir.AxisListType.X,
                                    op=mybir.AluOpType.max)
            nmx = dp.tile([GPC, 1], dt.float32, tag="nmx")
            nc.scalar.mul(nmx[:], mx[:], -1.0)
            e = dp.tile([GPC, 2], dt.float32, tag="e")
            nc.scalar.activation(e[:], lsb[:], AF.Exp, bias=nmx[:])
            s = dp.tile([GPC, 1], dt.float32, tag="s")
            nc.vector.tensor_reduce(s[:], e[:], axis=mybir.AxisListType.X,
                                    op=mybir.AluOpType.add)
            r = dp.tile([GPC, 1], dt.float32, tag="r")
            nc.vector.reciprocal(r[:], s[:])
            o = dp.tile([GPC, 2], dt.float32, tag="o")
            nc.vector.tensor_scalar_mul(o[:], e[:], r[:])
            nc.sync.dma_start(t_out.ap(), o[:])
    nc.compile()
    return nc


def _prep(x, W1, b1, W2, b2, W3, b3, Wc, bc, edge_index, batch):
    N = x.shape[0]
    G = int(batch.max()) + 1
    assert G % NC == 0, G
    GPC = G // NC
    src = np.asarray(edge_index[0], dtype=np.int64)
    dst = np.asarray(edge_index[1], dtype=np.int64)
    batch = np.asarray(batch, dtype=np.int64)
    assert np.all(np.diff(batch) >= 0), "batch must be sorted"

    deg = np.bincount(dst, minlength=N).astype(np.float64) + 1.0
    dinv = (1.0 / np.sqrt(deg)).astype(np.float32)

    gstart = np.searchsorted(batch, np.arange(G))
    gcnt = np.diff(np.append(gstart, N))
    assert gcnt.min() >= 1, "empty graph"
    SLOT = 240
    assert gcnt.max() <= SLOT, f"graph too large: {gcnt.max()}"
    R = GPC * SLOT                      # 15360 rows per core
    assert R % 128 == 0 and 2 * R < 32768
    ntiles = R // 128
    assert ntiles == WPS * NSP and ntiles % 15 == 0

    nodes = np.arange(N)
    rank_in_graph = nodes - gstart[batch]
    row_local = (batch % GPC) * SLOT + rank_in_graph      # row within owner core
    owner = batch // GPC

    # ---- per-core edge streams: sorted by (sp, bucket, window, src-row) ----
    e_owner = owner[dst]
    e_bkt = (owner[src] >> 1).astype(np.int64)
    e_dloc = row_local[dst]
    e_gloc = (owner[src] & 1) * R + row_local[src]
    e_w = e_dloc >> 7                     # dst window 0..ntiles-1

    # per-core counts per (sp, b, w)
    counts = np.zeros((NC, NSP, 4, WPS), np.int64)
    per_core = []
    for c in range(NC):
        m = e_owner == c
        b, w, dl, g = e_bkt[m], e_w[m], e_dloc[m], e_gloc[m]
        spv = w // WPS
        order = np.lexsort((g, w, b, spv))
        b, w, dl, g = b[order], w[order], dl[order], g[order]
        np.add.at(counts[c], (spv[order], b, w % WPS), 1)
        per_core.append((b, w, dl, g))

    cnt_max = counts.max(axis=0)                       # [NSP, 4, WPS]
    n_pad = ((cnt_max + 127) // 128) * 128
    n_pad = np.maximum(n_pad, 128)

    # schedule (uniform across cores)
    sched = []
    tok_off = 0
    col = 0
    for sp in range(NSP):
        spd = {"tok_off": tok_off, "insts": [], "tiles": []}
        ntok_sp = 0
        for b in range(4):
            io = ntok_sp                                # token offset within sp
            insts = []
            tiles = []
            # split this bucket's stream into <=CAP instructions
            total_b = int(n_pad[sp, b].sum())
            pos = 0
            while pos < total_b:
                n = min(CAP, total_b - pos)
                insts.append((io + pos, n))
                pos += n
            # tiles in stream order with window labels
            tpos = 0
            first_seen = set()
            for w in range(WPS):
                nt = int(n_pad[sp, b, w]) // 128
                for t in range(nt):
                    ii = tpos // CAP                    # instruction index
                    ofs = (tpos % CAP) // 128
                    first = (b == 0 and t == 0)
                    last = (b == 3 and t == nt - 1)
                    tiles.append((sp * WPS + w, col, ii, ofs, first, last))
                    col += 1
                    tpos += 128
            spd["insts"].append(insts)
            spd["tiles"].append(tiles)
            ntok_sp += total_b
        spd["ntok"] = ntok_sp
        tok_off += ntok_sp
        sched.append(spd)
    TOT = tok_off
    NTILE = col

    # per-core packed gidx + dpos
    gidx_all, dpos_all = [], []
    for c in range(NC):
        b, w, dl, g = per_core[c]
        spv = w // WPS
        wloc = w % WPS
        gi = np.zeros(TOT, np.int64)
        dp_ = np.full(NTILE * 128, -1.0, np.float32)
        # walk in the same (sp, b, w) order
        starts = {}
        o = 0
        for sp in range(NSP):
            for bb in range(4):
                for ww in range(WPS):
                    starts[(sp, bb, ww)] = o
                    o += int(n_pad[sp, bb, ww])
        tile_off = {}
        o = 0
        for sp in range(NSP):
            for bb in range(4):
                for ww in range(WPS):
                    tile_off[(sp, bb, ww)] = o
                    o += int(n_pad[sp, bb, ww])
        key = spv * (4 * WPS) + b * WPS + wloc
        order = np.argsort(key, kind="stable")   # already sorted; stable keeps g order
        ks = key[order]
        # positions within each (sp,b,w) run
        runstart = np.r_[0, np.flatnonzero(np.diff(ks)) + 1]
        runkey = ks[runstart]
        runlen = np.diff(np.append(runstart, len(ks)))
        for rk, rs, rl in zip(runkey, runstart, runlen):
            sp, rem = divmod(int(rk), 4 * WPS)
            bb, ww = divmod(rem, WPS)
            base = starts[(sp, bb, ww)]
            idxs = order[rs:rs + rl]
            gi[base:base + rl] = g[idxs]
            dp_[base:base + rl] = (dl[idxs] & 127).astype(np.float32)
        gidx_all.append(_pack_idx(gi))
        # dpos: token position base == tile column base * 128 (same ordering)
        dpos_all.append(np.ascontiguousarray(
            dp_.reshape(NTILE, 128).T))

    # ---- dense-phase per-core data ----
    import concourse.mybir as _mb
    BF16 = _mb.dt.np(_mb.dt.bfloat16)
    W1p = np.zeros((4, F), np.float32); W1p[:3, :W1.shape[1]] = W1
    W2p = np.zeros((F, F), np.float32); W2p[:W2.shape[0], :] = W2
    W3p = np.asarray(W3, np.float32)
    W1p = W1p.astype(BF16); W2p = W2p.astype(BF16); W3p = W3p.astype(BF16)
    Wcp = np.asarray(Wc, np.float32)
    b1p = np.zeros(F, np.float32); b1p[:b1.shape[0]] = b1
    b2p = np.asarray(b2, np.float32)
    b3p = np.asarray(b3, np.float32)
    ident = np.eye(128, dtype=np.float32)
    iota = np.tile(np.arange(128, dtype=np.float32)[None, :], (128, 1))

    in_maps = []
    for c in range(NC):
        sel_nodes = nodes[owner == c]
        rl = row_local[sel_nodes]
        xnm = np.zeros((R, 4), np.float32)
        xnm[rl, :3] = np.asarray(x, np.float32)[sel_nodes]
        dv = np.zeros(R, np.float32)
        dv[rl] = dinv[sel_nodes]
        valid = np.zeros(R, np.float32)
        valid[rl] = 1.0
        cnt_c = gcnt[c * GPC:(c + 1) * GPC].astype(np.float32)
        sel = np.zeros((128, 15, 8), np.float32)
        rows = (np.arange(15 * 128)).reshape(15, 128)
        slot_in_grp = rows // SLOT
        for j in range(15):
            sel[np.arange(128), j, slot_in_grp[j]] = 1.0
        cntinv = np.tile((1.0 / cnt_c)[None, :], (128, 1)).astype(np.float32)
        mask = valid
        negm = ((1.0 - valid) * NEG_BIG).astype(np.float32)
        in_maps.append({
            "xnm": xnm.reshape(ntiles, 128, 4).transpose(1, 0, 2).copy(),
            "gidx": gidx_all[c],
            "dpos": dpos_all[c],
            "dinv": dv.reshape(ntiles, 128).T.copy(),
            "mask": mask.reshape(ntiles, 128).T.copy(),
            "negm": negm.reshape(ntiles, 128).T.copy(),
            "sel": sel,
            "cntinv": cntinv,
            "iota": iota,
            "w1": W1p, "w2": W2p, "w3": W3p, "wc": Wcp,
            "b1r": np.tile(b1p, (128, 1)),
            "b2r": np.tile(b2p, (128, 1)),
            "b3r": np.tile(b3p, (128, 1)),
            "bcr": np.tile(np.asarray(bc, np.float32), (GPC, 1)),
            "ident": ident,
        })
    cfg = dict(R=R, ntiles=ntiles, sched=sched, GPC=GPC, SLOT=SLOT, TOT=TOT,
               NTILE=NTILE)
    return in_maps, cfg


def kernel(x, W1, b1, W2, b2, W3, b3, Wc, bc, edge_index, batch):
    in_maps, cfg = _prep(x, W1, b1, W2, b2, W3, b3, Wc, bc, edge_index, batch)
    GPC, SLOT = cfg["GPC"], cfg["SLOT"]
    nc = _build_nc(cfg["R"], cfg["ntiles"], cfg["sched"], GPC, SLOT,
                   cfg["TOT"], cfg["NTILE"])
    import os as _os
    _trace = _os.environ.get("GCN_TRACE", "0") == "1"
    res = run_bass_kernel_spmd(nc, in_maps, list(range(NC)), trace=_trace)
    global LAST_EXEC_NS, LAST_RESULT
    LAST_EXEC_NS = res.exec_time_ns
    LAST_RESULT = res
    outs = []
    for c in range(NC):
        o = res.results[c]["out"][:GPC].astype(np.float32)
        outs.append(o)
    return np.concatenate(outs, axis=0)


# revision 12
# speedup vs baseline: 6.4448x; 2.2984x over previous
"""GCN (3-layer) + graph pooling kernel for 8 Trainium2 NeuronCores.

Sharding: each core owns 64 of the 512 graphs (contiguous node range, since
`batch` is sorted). Every graph gets a fixed SLOT=240-row slot so per-core
layouts are uniform (SPMD). Edges are assigned to the core owning their dst.

Edge phase: per-edge dma_gather (256B rows) from the AllGather'd table on 4
rotating SWDGE queues (ring-drain parallelism ~2.3ns/token), aggregated
on-chip: edges sorted by (super-phase, src-bucket, dst-window); per 128-token
tile a one-hot S matrix (DVE iota==dpos compare) routes messages into a PSUM
window accumulator via TensorE matmul. No dma_scatter_add at all. Self-loop
terms are added from the local bounce buffer during the PSUM flush; the agg
table lives in SBUF (bf16) for the next dense phase.
"""
import numpy as np

import concourse.bacc as bacc
import concourse.bass as bass
import concourse.mybir as mybir
import concourse.tile as tile
from concourse.bass_utils import run_bass_kernel_spmd

LAST_EXEC_NS = None
LAST_RESULT = None

F = 64          # padded feature width (layer1's 32 is zero-padded to 64)
NC = 8          # cores
NEG_BIG = -1.0e30
WPS = 15        # dst windows (128 rows) per super-phase
NSP = 8         # super-phases (WPS*NSP*128 == R)
CAP = 1024      # max tokens per gather instruction
NQ = 4          # SWDGE queues


def _pack_idx(flat: np.ndarray) -> np.ndarray:
    """token i -> [i % 16, i // 16], replicated down to 128 partitions."""
    t = len(flat)
    a = np.ascontiguousarray(flat.astype(np.int16).reshape(t // 16, 16).T)
    return np.tile(a, (8, 1))


def _build_nc(R, ntiles, sched, GPC, SLOT, TOT, NTILE):
    import os as _os
    PHASES = int(_os.environ.get("GCN_PHASES", "3"))  # 1=gather only, 2=+convert, 3=full
    nc = bacc.Bacc("TRN2", target_bir_lowering=False, debug=False,
                   num_devices=NC, dynamic_dma_scratch_size=65536,
                   num_swdge_queues=NQ)
    dt = mybir.dt
    AF = mybir.ActivationFunctionType

    # ---- external inputs ----
    t_xnm = nc.dram_tensor("xnm", [128, ntiles, 4], dt.float32, kind="ExternalInput")
    t_gidx = nc.dram_tensor("gidx", [128, TOT // 16], dt.int16, kind="ExternalInput")
    t_dpos = nc.dram_tensor("dpos", [128, NTILE], dt.float32, kind="ExternalInput")
    t_dinv = nc.dram_tensor("dinv", [128, ntiles], dt.float32, kind="ExternalInput")
    t_mask = nc.dram_tensor("mask", [128, ntiles], dt.float32, kind="ExternalInput")
    t_negm = nc.dram_tensor("negm", [128, ntiles], dt.float32, kind="ExternalInput")
    t_sel = nc.dram_tensor("sel", [128, 15, 8], dt.float32, kind="ExternalInput")
    t_cntinv = nc.dram_tensor("cntinv", [128, GPC], dt.float32, kind="ExternalInput")
    t_iota = nc.dram_tensor("iota", [128, 128], dt.float32, kind="ExternalInput")
    t_w1 = nc.dram_tensor("w1", [4, F], dt.bfloat16, kind="ExternalInput")
    t_w2 = nc.dram_tensor("w2", [F, F], dt.bfloat16, kind="ExternalInput")
    t_w3 = nc.dram_tensor("w3", [F, F], dt.bfloat16, kind="ExternalInput")
    t_wc = nc.dram_tensor("wc", [128, 2], dt.float32, kind="ExternalInput")
    t_b1 = nc.dram_tensor("b1r", [128, F], dt.float32, kind="ExternalInput")
    t_b2 = nc.dram_tensor("b2r", [128, F], dt.float32, kind="ExternalInput")
    t_b3 = nc.dram_tensor("b3r", [128, F], dt.float32, kind="ExternalInput")
    t_bc = nc.dram_tensor("bcr", [GPC, 2], dt.float32, kind="ExternalInput")
    t_id = nc.dram_tensor("ident", [128, 128], dt.float32, kind="ExternalInput")
    t_out = nc.dram_tensor("out", [GPC, 2], dt.float32, kind="ExternalOutput")

    # ---- internal DRAM ----
    t_bounce = [nc.dram_tensor(f"bounce{l}", [R, F], dt.float32) for l in range(3)]
    t_pfull = [nc.dram_tensor(f"pfull{l}", [NC * R, F], dt.float32,
                              addr_space="Shared") for l in range(3)]

    with tile.TileContext(nc) as tc:
        with (
            tc.tile_pool(name="persist", bufs=1) as pc,
            tc.tile_pool(name="dense", bufs=3) as dp,
            tc.tile_pool(name="idxp", bufs=2) as ip,
            tc.tile_pool(name="flush", bufs=2) as fl,
            tc.tile_pool(name="gat", bufs=10) as gp,
            tc.tile_pool(name="msg", bufs=10) as mp,
            tc.tile_pool(name="sgen", bufs=12) as sp_,
            tc.tile_pool(name="psum", bufs=2, space="PSUM") as pp,
            tc.tile_pool(name="psagg", bufs=1, space="PSUM") as pa,
        ):
            # ---- persistent tiles ----
            xnm_sb = pc.tile([128, ntiles, 4], dt.float32, tag="xnm")
            dinv_sb = pc.tile([128, ntiles], dt.float32, tag="dinv")
            mask_sb = pc.tile([128, ntiles], dt.float32, tag="mask")
            negm_sb = pc.tile([128, ntiles], dt.float32, tag="negm")
            sel_sb = pc.tile([128, 15, 8], dt.float32, tag="sel")
            cntinv_sb = pc.tile([128, GPC], dt.float32, tag="cntinv")
            iota_sb = pc.tile([128, 128], dt.float32, tag="iota")
            dpos_sb = pc.tile([128, NTILE], dt.float32, tag="dpos")
            w1_sb = pc.tile([4, F], dt.bfloat16, tag="w1")
            w2_sb = pc.tile([F, F], dt.bfloat16, tag="w2")
            w3_sb = pc.tile([F, F], dt.bfloat16, tag="w3")
            wc_sb = pc.tile([128, 2], dt.float32, tag="wc")
            b1_sb = pc.tile([128, F], dt.float32, tag="b1")
            b2_sb = pc.tile([128, F], dt.float32, tag="b2")
            b3_sb = pc.tile([128, F], dt.float32, tag="b3")
            bc_sb = pc.tile([GPC, 2], dt.float32, tag="bc")
            id_sb = pc.tile([128, 128], dt.float32, tag="ident")
            agg_sb = pc.tile([128, ntiles, F], dt.bfloat16, tag="agg")
            h3maxT = pc.tile([F, R], dt.float32, tag="h3maxT")
            combT = pc.tile([128, GPC], dt.float32, tag="combT")

            nc.sync.dma_start(xnm_sb[:], t_xnm.ap())
            nc.sync.dma_start(dinv_sb[:], t_dinv.ap())
            nc.sync.dma_start(mask_sb[:], t_mask.ap())
            nc.sync.dma_start(negm_sb[:], t_negm.ap())
            nc.sync.dma_start(sel_sb[:], t_sel.ap())
            nc.sync.dma_start(cntinv_sb[:], t_cntinv.ap())
            nc.sync.dma_start(iota_sb[:], t_iota.ap())
            nc.sync.dma_start(dpos_sb[:], t_dpos.ap())
            nc.sync.dma_start(w1_sb[:], t_w1.ap())
            nc.sync.dma_start(w2_sb[:], t_w2.ap())
            nc.sync.dma_start(w3_sb[:], t_w3.ap())
            nc.sync.dma_start(wc_sb[:], t_wc.ap())
            nc.sync.dma_start(b1_sb[:], t_b1.ap())
            nc.sync.dma_start(b2_sb[:], t_b2.ap())
            nc.sync.dma_start(b3_sb[:], t_b3.ap())
            nc.sync.dma_start(bc_sb[:], t_bc.ap())
            nc.sync.dma_start(id_sb[:], t_id.ap())

            w_for = {1: w2_sb, 2: w3_sb}
            b_for = {1: b1_sb, 2: b2_sb}
            qctr = [0]

            # ---- dense tile: compute ps row-tile m of layer l, write bounce[l]
            def dense_tile(l, m):
                ts = slice(m * 128, (m + 1) * 128)
                dcol = dinv_sb[:, m:m + 1]
                if l == 0:
                    xtp = pp.tile([F, 128], dt.float32, tag="ztp")
                    nc.tensor.transpose(xtp[0:4, :], xnm_sb[:, m, :], id_sb[:])
                    xts = dp.tile([F, 128], dt.bfloat16, tag="zts")
                    nc.scalar.copy(xts[0:4, :], xtp[0:4, :])
                    mm = pp.tile([128, F], dt.float32, tag="mm")
                    nc.tensor.matmul(mm[:], xts[0:3, :], w1_sb[0:3, :],
                                     start=True, stop=True)
                else:
                    z = dp.tile([128, F], dt.float32, tag="z")
                    nc.vector.tensor_scalar_mul(z[:], agg_sb[:, m, :], dcol)
                    nc.vector.tensor_add(z[:], z[:], b_for[l][:])
                    zr = dp.tile([128, F], dt.float32, tag="zr")
                    nc.scalar.activation(zr[:], z[:], AF.Relu)
                    ztp = pp.tile([F, 128], dt.float32, tag="ztp")
                    nc.tensor.transpose(ztp[:], zr[:], id_sb[:])
                    zts = dp.tile([F, 128], dt.bfloat16, tag="zts")
                    nc.scalar.copy(zts[:], ztp[:])
                    mm = pp.tile([128, F], dt.float32, tag="mm")
                    nc.tensor.matmul(mm[:], zts[:], w_for[l][:],
                                     start=True, stop=True)
                ps = dp.tile([128, F], dt.float32, tag="ps")
                nc.scalar.activation(ps[:], mm[:], AF.Copy, scale=dcol)
                nc.sync.dma_start(t_bounce[l].ap()[ts], ps[:])

            def allgather(l):
                nc.gpsimd.collective_compute(
                    "AllGather", mybir.AluOpType.bypass,
                    replica_groups=[list(range(NC))],
                    ins=[t_bounce[l].ap().opt()],
                    outs=[t_pfull[l].ap().opt()],
                )

            # ---- pooling tile (layer-2 output): gmean matmul + max-pool stage
            def pool_tile(m):
                ts = slice(m * 128, (m + 1) * 128)
                h = dp.tile([128, F], dt.float32, tag="z")
                nc.vector.tensor_scalar_mul(h[:], agg_sb[:, m, :],
                                            dinv_sb[:, m:m + 1])
                nc.vector.tensor_add(h[:], h[:], b3_sb[:])
                hs = dp.tile([128, F], dt.float32, tag="hs")
                nc.vector.tensor_scalar_mul(hs[:], h[:], mask_sb[:, m:m + 1])
                g = m // 15
                nc.tensor.matmul(gmean[:, g * 8:(g + 1) * 8], hs[:],
                                 sel_sb[:, m % 15, :],
                                 start=(m % 15 == 0), stop=(m % 15 == 14))
                hm = dp.tile([128, F], dt.float32, tag="zr")
                nc.vector.tensor_scalar(hm[:], h[:], mask_sb[:, m:m + 1],
                                        negm_sb[:, m:m + 1],
                                        op0=mybir.AluOpType.mult,
                                        op1=mybir.AluOpType.add)
                htp = pp.tile([F, 128], dt.float32, tag="ztp")
                nc.tensor.transpose(htp[:], hm[:], id_sb[:])
                nc.scalar.copy(h3maxT[:, ts], htp[:])

            # ---- edge phase of layer l; folds dense of l+1 (or pooling) per sp
            def edge_layer(l):
                for sp in range(NSP):
                    spd = sched[sp]
                    psw = pa.tile([128, WPS, F], dt.float32, tag="psw")
                    idx_sb = ip.tile([128, spd["ntok"] // 16], dt.int16, tag="gi")
                    o16 = spd["tok_off"] // 16
                    nc.sync.dma_start(idx_sb[:],
                                      t_gidx.ap()[:, o16:o16 + spd["ntok"] // 16])
                    for b in range(4):
                        src = t_pfull[l].ap()[b * 2 * R:(b + 1) * 2 * R]
                        gtbs = []
                        for (io, n) in spd["insts"][b]:
                            gt = gp.tile([128, CAP // 128, F], dt.float32, tag="gt")
                            nc.gpsimd.dma_gather(
                                gt[:, :n // 128, :], src,
                                idx_sb[:, io // 16:io // 16 + n // 16], n, n, F,
                                queue_num=qctr[0] % NQ)
                            qctr[0] += 1
                            gtb = mp.tile([128, CAP // 128, F], dt.bfloat16, tag="gtb")
                            if PHASES >= 2:
                                nc.scalar.copy(gtb[:, :n // 128, :], gt[:, :n // 128, :])
                            gtbs.append(gtb)
                        for (w, col, ii, ofs, first, last) in (spd["tiles"][b] if PHASES >= 3 else []):
                            S = sp_.tile([128, 128], dt.bfloat16, tag="S")
                            nc.vector.tensor_scalar(
                                S[:], iota_sb[:], dpos_sb[:, col:col + 1], None,
                                op0=mybir.AluOpType.is_equal)
                            nc.tensor.matmul(psw[:, w - sp * WPS, :], S[:],
                                             gtbs[ii][:, ofs, :],
                                             start=first, stop=last)
                    # flush: agg = psw + self-loop rows from bounce
                    selfsb = fl.tile([128, WPS, F], dt.float32, tag="selfsb")
                    rs = sp * WPS * 128
                    nc.sync.dma_start(
                        selfsb[:],
                        t_bounce[l].ap()[rs:rs + WPS * 128].rearrange(
                            "(t p) f -> p t f", p=128))
                    if PHASES >= 3:
                        nc.vector.tensor_add(agg_sb[:, sp * WPS:(sp + 1) * WPS, :],
                                             psw[:], selfsb[:])
                    else:
                        nc.vector.tensor_copy(
                            agg_sb[:, sp * WPS:(sp + 1) * WPS, :], selfsb[:])
                    # fold next layer's dense (or pooling) for this sp's tiles
                    for m in range(sp * WPS, (sp + 1) * WPS):
                        if l < 2:
                            dense_tile(l + 1, m)
                        else:
                            pool_tile(m)

            gmean = pa.tile([F, GPC], dt.float32, tag="gmean")
            for m in range(ntiles):
                dense_tile(0, m)
            allgather(0)
            for l in range(3):
                edge_layer(l)
                if l < 2:
                    allgather(l + 1)

            # ---------------- pooling finalize + classifier ----------------
            nc.vector.tensor_reduce(
                combT[0:F, :], h3maxT[:].rearrange("p (g s) -> p g s", s=SLOT),
                axis=mybir.AxisListType.X, op=mybir.AluOpType.max)
            nc.scalar.copy(combT[F:128, :], gmean[:])
            nc.vector.tensor_mul(combT[F:128, :], combT[F:128, :], cntinv_sb[F:128, :])
            logits = pa.tile([GPC, 2], dt.float32, tag="logits")
            nc.tensor.matmul(logits[:], combT[:], wc_sb[:], start=True, stop=True)
            lsb = dp.tile([GPC, 2], dt.float32, tag="lsb")
            nc.scalar.copy(lsb[:], logits[:])
            nc.vector.tensor_add(lsb[:], lsb[:], bc_sb[:])
            mx = dp.tile([GPC, 1], dt.float32, tag="mx")
            nc.vector.tensor_reduce(mx[:], lsb[:], axis=mybir.AxisListType.X,
                                    op=mybir.AluOpType.max)
            nmx = dp.tile([GPC, 1], dt.float32, tag="nmx")
            nc.scalar.mul(nmx[:], mx[:], -1.0)
            e = dp.tile([GPC, 2], dt.float32, tag="e")
            nc.scalar.activation(e[:], lsb[:], AF.Exp, bias=nmx[:])
            s = dp.tile([GPC, 1], dt.float32, tag="s")
            nc.vector.tensor_reduce(s[:], e[:], axis=mybir.AxisListType.X,
                                    op=mybir.AluOpType.add)
            r = dp.tile([GPC, 1], dt.float32, tag="r")
            nc.vector.reciprocal(r[:], s[:])
            o = dp.tile([GPC, 2], dt.float32, tag="o")
            nc.vector.tensor_scalar_mul(o[:], e[:], r[:])
            nc.sync.dma_start(t_out.ap(), o[:])
    nc.compile()
    return nc


def _prep(x, W1, b1, W2, b2, W3, b3, Wc, bc, edge_index, batch):
    N = x.shape[0]
    G = int(batch.max()) + 1
    assert G % NC == 0, G
    GPC = G // NC
    src = np.asarray(edge_index[0], dtype=np.int64)
    dst = np.asarray(edge_index[1], dtype=np.int64)
    batch = np.asarray(batch, dtype=np.int64)
    assert np.all(np.diff(batch) >= 0), "batch must be sorted"

    deg = np.bincount(dst, minlength=N).astype(np.float64) + 1.0
    dinv = (1.0 / np.sqrt(deg)).astype(np.float32)

    gstart = np.searchsorted(batch, np.arange(G))
    gcnt = np.diff(np.append(gstart, N))
    assert gcnt.min() >= 1, "empty graph"
    SLOT = 240
    assert gcnt.max() <= SLOT, f"graph too large: {gcnt.max()}"
    R = GPC * SLOT                      # 15360 rows per core
    assert R % 128 == 0 and 2 * R < 32768
    ntiles = R // 128
    assert ntiles == WPS * NSP and ntiles % 15 == 0

    nodes = np.arange(N)
    rank_in_graph = nodes - gstart[batch]
    row_local = (batch % GPC) * SLOT + rank_in_graph      # row within owner core
    owner = batch // GPC

    # ---- per-core edge streams: sorted by (sp, bucket, window, src-row) ----
    e_owner = owner[dst]
    e_bkt = (owner[src] >> 1).astype(np.int64)
    e_dloc = row_local[dst]
    e_gloc = (owner[src] & 1) * R + row_local[src]
    e_w = e_dloc >> 7                     # dst window 0..ntiles-1

    # per-core counts per (sp, b, w)
    counts = np.zeros((NC, NSP, 4, WPS), np.int64)
    per_core = []
    for c in range(NC):
        m = e_owner == c
        b, w, dl, g = e_bkt[m], e_w[m], e_dloc[m], e_gloc[m]
        spv = w // WPS
        order = np.lexsort((g, w, b, spv))
        b, w, dl, g = b[order], w[order], dl[order], g[order]
        np.add.at(counts[c], (spv[order], b, w % WPS), 1)
        per_core.append((b, w, dl, g))

    cnt_max = counts.max(axis=0)                       # [NSP, 4, WPS]

    # schedule (uniform across cores); window runs are NOT 128-padded —
    # boundary tiles get one (S, matmul) part per window they span.
    sched = []
    tok_off = 0
    col = 0
    starts = {}
    for sp in range(NSP):
        spd = {"tok_off": tok_off, "insts": [], "tiles": []}
        ntok_sp = 0
        for b in range(4):
            io = ntok_sp
            runs = []                       # (w, start_within_bucket, len)
            pos = 0
            for w in range(WPS):
                n = int(cnt_max[sp, b, w])
                starts[(sp, b, w)] = tok_off + io + pos
                runs.append((w, pos, n))
                pos += n
            total_b = ((pos + 127) // 128) * 128
            # instructions
            insts = []
            ipos = 0
            while ipos < total_b:
                n = min(CAP, total_b - ipos)
                insts.append((io + ipos, n))
                ipos += n
            # tile parts: for each 128-token tile, one entry per window present
            tiles = []
            parts_by_w = {}
            for (w, rs, rl) in runs:
                if rl == 0:
                    continue
                t0, t1 = rs // 128, (rs + rl - 1) // 128
                for t in range(t0, t1 + 1):
                    ii = (t * 128) // CAP
                    ofs = (t % (CAP // 128))
                    entry = [sp * WPS + w, col, ii, ofs, False, False,
                             b, t, rs, rl]
                    tiles.append(entry)
                    parts_by_w.setdefault(w, []).append(entry)
                    col += 1
            spd["insts"].append(insts)
            spd["tiles"].append(tiles)
            ntok_sp += total_b
        # start/stop flags per window across buckets (program order: b, tile)
        byw = {}
        for b in range(4):
            for e in spd["tiles"][b]:
                byw.setdefault(e[0], []).append(e)
        for w, es in byw.items():
            es[0][4] = True
            es[-1][5] = True
        # strip helper fields
        for b in range(4):
            spd["tiles"][b] = [tuple(e[:6]) for e in spd["tiles"][b]]
        spd["ntok"] = ntok_sp
        tok_off += ntok_sp
        sched.append(spd)
    TOT = tok_off
    NTILE = col

    # per-core packed gidx + dpos
    gidx_all, dpos_all = [], []
    # rebuild per-part info for dpos fill
    part_info = []                  # (col, global_tile_base, w, run_start, run_len)
    colc = 0
    for sp in range(NSP):
        base_sp = sched[sp]["tok_off"]
        ntok_sp2 = 0
        for b in range(4):
            pos = 0
            runs = []
            for w in range(WPS):
                n = int(cnt_max[sp, b, w])
                runs.append((w, pos, n))
                pos += n
            total_b = ((pos + 127) // 128) * 128
            for (w, rs, rl) in runs:
                if rl == 0:
                    continue
                t0, t1 = rs // 128, (rs + rl - 1) // 128
                for t in range(t0, t1 + 1):
                    part_info.append((colc, base_sp + ntok_sp2 + t * 128,
                                      w, base_sp + ntok_sp2 + rs, rl))
                    colc += 1
            ntok_sp2 += total_b
    assert colc == NTILE

    for c in range(NC):
        b, w, dl, g = per_core[c]
        spv = w // WPS
        wloc = w % WPS
        gi = np.zeros(TOT, np.int64)
        dfull = np.full(TOT, -1.0, np.float32)   # dpos by token position
        key = spv * (4 * WPS) + b * WPS + wloc
        ks = key                                  # already sorted
        runstart = np.r_[0, np.flatnonzero(np.diff(ks)) + 1]
        runkey = ks[runstart]
        runlen = np.diff(np.append(runstart, len(ks)))
        for rk, rs_, rl_ in zip(runkey, runstart, runlen):
            sp, rem = divmod(int(rk), 4 * WPS)
            bb, ww = divmod(rem, WPS)
            base = starts[(sp, bb, ww)]
            gi[base:base + rl_] = g[rs_:rs_ + rl_]
            dfull[base:base + rl_] = (dl[rs_:rs_ + rl_] & 127).astype(np.float32)
        gidx_all.append(_pack_idx(gi))
        # dpos columns per part: tile window [tb, tb+128) ∩ run [rstart, rstart+rlen)
        dp_ = np.full((128, NTILE), -1.0, np.float32)
        for (cc, tb, ww, rstart, rlen) in part_info:
            lo = max(tb, rstart)
            hi = min(tb + 128, rstart + rlen)
            if hi > lo:
                seg = dfull[lo:hi].copy()
                dp_[lo - tb:hi - tb, cc] = seg
        dpos_all.append(np.ascontiguousarray(dp_))

    # ---- dense-phase per-core data ----
    import concourse.mybir as _mb
    BF16 = _mb.dt.np(_mb.dt.bfloat16)
    W1p = np.zeros((4, F), np.float32); W1p[:3, :W1.shape[1]] = W1
    W2p = np.zeros((F, F), np.float32); W2p[:W2.shape[0], :] = W2
    W3p = np.asarray(W3, np.float32)
    W1p = W1p.astype(BF16); W2p = W2p.astype(BF16); W3p = W3p.astype(BF16)
    Wcp = np.asarray(Wc, np.float32)
    b1p = np.zeros(F, np.float32); b1p[:b1.shape[0]] = b1
    b2p = np.asarray(b2, np.float32)
    b3p = np.asarray(b3, np.float32)
    ident = np.eye(128, dtype=np.float32)
    iota = np.tile(np.arange(128, dtype=np.float32)[None, :], (128, 1))

    in_maps = []
    for c in range(NC):
        sel_nodes = nodes[owner == c]
        rl = row_local[sel_nodes]
        xnm = np.zeros((R, 4), np.float32)
        xnm[rl, :3] = np.asarray(x, np.float32)[sel_nodes]
        dv = np.zeros(R, np.float32)
        dv[rl] = dinv[sel_nodes]
        valid = np.zeros(R, np.float32)
        valid[rl] = 1.0
        cnt_c = gcnt[c * GPC:(c + 1) * GPC].astype(np.float32)
        sel = np.zeros((128, 15, 8), np.float32)
        rows = (np.arange(15 * 128)).reshape(15, 128)
        slot_in_grp = rows // SLOT
        for j in range(15):
            sel[np.arange(128), j, slot_in_grp[j]] = 1.0
        cntinv = np.tile((1.0 / cnt_c)[None, :], (128, 1)).astype(np.float32)
        mask = valid
        negm = ((1.0 - valid) * NEG_BIG).astype(np.float32)
        in_maps.append({
            "xnm": xnm.reshape(ntiles, 128, 4).transpose(1, 0, 2).copy(),
            "gidx": gidx_all[c],
            "dpos": dpos_all[c],
            "dinv": dv.reshape(ntiles, 128).T.copy(),
            "mask": mask.reshape(ntiles, 128).T.copy(),
            "negm": negm.reshape(ntiles, 128).T.copy(),
            "sel": sel,
            "cntinv": cntinv,
            "iota": iota,
            "w1": W1p, "w2": W2p, "w3": W3p, "wc": Wcp,
            "b1r": np.tile(b1p, (128, 1)),
            "b2r": np.tile(b2p, (128, 1)),
            "b3r": np.tile(b3p, (128, 1)),
            "bcr": np.tile(np.asarray(bc, np.float32), (GPC, 1)),
            "ident": ident,
        })
    cfg = dict(R=R, ntiles=ntiles, sched=sched, GPC=GPC, SLOT=SLOT, TOT=TOT,
               NTILE=NTILE)
    return in_maps, cfg


def kernel(x, W1, b1, W2, b2, W3, b3, Wc, bc, edge_index, batch):
    in_maps, cfg = _prep(x, W1, b1, W2, b2, W3, b3, Wc, bc, edge_index, batch)
    GPC, SLOT = cfg["GPC"], cfg["SLOT"]
    nc = _build_nc(cfg["R"], cfg["ntiles"], cfg["sched"], GPC, SLOT,
                   cfg["TOT"], cfg["NTILE"])
    import os as _os
    _trace = _os.environ.get("GCN_TRACE", "0") == "1"
    res = run_bass_kernel_spmd(nc, in_maps, list(range(NC)), trace=_trace)
    global LAST_EXEC_NS, LAST_RESULT
    LAST_EXEC_NS = res.exec_time_ns
    LAST_RESULT = res
    outs = []
    for c in range(NC):
        o = res.results[c]["out"][:GPC].astype(np.float32)
        outs.append(o)
    return np.concatenate(outs, axis=0)
